# revision 4
# baseline (speedup 1.0000x reference)
"""Causal sliding-window GQA with 3D RoPE on 8 Trainium2 NeuronCores.

Sharding: tensor-parallel over the 16 query heads (2 per core); each core
also computes its group's kv head (duplicated across the 2 cores sharing
it). o_proj is row-parallel; the 8 partial outputs are summed on host.

Device kernel (per core, SPMD-uniform; all per-core differences are data):
  - Q/K/V projections as fp32r matmuls from host-pretransposed hidden^T.
  - 3D RoPE applied on-device with host-precomputed cos/sin tables; head
    dims are permuted [evens|odds] host-side (consistent for Q and K, so
    scores are unchanged) to make the rotation block-contiguous.
  - Scores are built transposed, S^T[k, q], so the exp output (ACT,
    PSUM->SBUF) feeds the P^T @ [V|1] matmul directly; the appended ones
    column yields the softmax denominators for free.
  - The sliding-window + frame-causal mask is precompiled host-side into a
    block-sparse live-tile list: fully masked key tiles are skipped, fully
    visible tiles run unmasked, and only partial tiles get an additive
    -1e30 bias region.
"""

import os
import sys

import numpy as np

import concourse.bass as bass
import concourse.tile as tile
from concourse import mybir
from concourse.bass_utils import run_bass_kernel_spmd

sys.path.insert(0, os.path.dirname(os.path.abspath(__file__)))
from kernel_build import build as _build_prog  # noqa: E402

F32 = mybir.dt.float32
F32R = mybir.dt.float32r
AF = mybir.ActivationFunctionType
ALU = mybir.AluOpType

E = 1024
H = 16
KVH = 4
D = 64
HALF = 32
WINDOW = 32
SCALE = 0.125  # 1/sqrt(D)
NEG = np.float32(-1.0e30)
NCORES = 8

PERM = np.concatenate([np.arange(0, D, 2), np.arange(1, D, 2)])  # [evens|odds]

LAST_EXEC_TIME_NS = None


def kernel(hidden_states, past_k, past_v, wq, wk, wv, wo,
           new_t, new_d, new_b, past_t, past_d, past_b):
    global LAST_EXEC_TIME_NS
    hidden_states = np.asarray(hidden_states)
    B, Sn, _ = hidden_states.shape
    assert B == 1 and Sn % 128 == 0
    past_k = np.asarray(past_k, dtype=np.float32)
    past_v = np.asarray(past_v, dtype=np.float32)
    wq = np.asarray(wq, dtype=np.float32)
    wk = np.asarray(wk, dtype=np.float32)
    wv = np.asarray(wv, dtype=np.float32)
    wo = np.asarray(wo, dtype=np.float32)
    new_t = np.asarray(new_t)
    past_t = np.asarray(past_t)
    Lp = past_t.shape[0]

    # ---- mask structure (host) ----
    min_time = int(new_t.max()) - (WINDOW - 1)
    keep = int(np.searchsorted(past_t, min_time, side="left"))
    KP = Lp - keep
    KPp = ((KP + 127) // 128) * 128
    k0p = Lp - KPp
    pt_kept = past_t[k0p:Lp].astype(np.int64)
    sq_past = np.searchsorted(new_t, pt_kept, side="left").astype(np.int64)
    sq_past[pt_kept < min_time] = Sn  # pad keys: never visible
    sq_new = np.searchsorted(new_t, new_t, side="left").astype(np.int64)

    live = []  # (src, tile_idx, qlo, qpe, bias_off)
    mats = []
    boff = 0
    tiles = [("past", t, sq_past[t * 128 : (t + 1) * 128]) for t in range(KPp // 128)]
    tiles += [("new", t, sq_new[t * 128 : (t + 1) * 128]) for t in range(Sn // 128)]
    tiles_live = []
    for src, t, s in tiles:
        qlo = int(s.min())
        if qlo >= Sn:
            continue
        qpe = min(int(s.max()), Sn)
        tiles_live.append((src, t, qlo, qpe, s))
    tiles_live.sort(key=lambda x: x[2])
    assert tiles_live and tiles_live[0][2] == 0
    for src, t, qlo, qpe, s in tiles_live:
        w = qpe - qlo
        if w > 0:
            m = np.where(
                s[:, None] <= np.arange(qlo, qpe)[None, :], np.float32(0.0), NEG
            ).astype(np.float32)
            mats.append(m)
            live.append((src, t, qlo, qpe, boff))
            boff += w
        else:
            live.append((src, t, qlo, qpe, 0))
    BW = max(boff, 1)
    biases = np.zeros((128, BW), np.float32)
    if mats:
        biases[:, 0 : sum(m.shape[1] for m in mats)] = np.concatenate(mats, axis=1)

    # ---- RoPE trig tables (host, mirroring reference fp32 order) ----
    dim_idx = np.arange(HALF, dtype=np.float32)
    inv = (1.0 / (10000.0 ** (2.0 * dim_idx / D))).astype(np.float32)
    f_d = np.float32(10000.0 / 100.0)
    f_b = np.float32(10000.0 / 100.0)
    ang = (
        new_t.astype(np.float32)[:, None] * inv
        + new_d.astype(np.float32)[:, None] * inv * f_d
        + new_b.astype(np.float32)[:, None] * inv * f_b
    ).astype(np.float32)
    cos = np.cos(ang).astype(np.float32).T  # [32, Sn]
    sin = np.sin(ang).astype(np.float32).T
    t3 = np.concatenate([cos, sin, cos, sin], axis=0)  # [128, Sn]
    t4 = np.concatenate([sin, cos, sin, cos], axis=0)

    # ---- per-core inputs ----
    hiddenT = np.ascontiguousarray(hidden_states[0].T.astype(np.float32))
    ident = np.eye(128, dtype=np.float32)
    KPa = max(KPp, 128)
    in_maps = []
    for c in range(NCORES):
        g = c // 2
        h0 = 4 * g + 2 * (c % 2)
        h1 = h0 + 1
        wq_cols = np.concatenate([h0 * D + PERM, h1 * D + PERM])
        wqp = np.ascontiguousarray(wq[:, wq_cols])
        wkv = np.ascontiguousarray(
            np.concatenate(
                [wk[:, g * D + PERM], wv[:, g * D : (g + 1) * D]], axis=1
            )
        )
        wor = np.ascontiguousarray(
            np.concatenate([wo[h0 * D : (h0 + 1) * D, :], wo[h1 * D : (h1 + 1) * D, :]])
        )
        pkT = np.zeros((D, KPa), np.float32)
        pvm = np.zeros((KPa, D), np.float32)
        if KPp:
            pkT[:, 0:KPp] = past_k[0, g, k0p:Lp, :][:, PERM].T
            pvm[0:KPp] = past_v[0, g, k0p:Lp, :]
        in_maps.append(
            dict(
                hT=hiddenT, wqp=wqp, wkv=wkv, wor=wor,
                pkT=np.ascontiguousarray(pkT), pv=np.ascontiguousarray(pvm),
                t3=t3, t4=t4, bias=biases, ident=ident,
            )
        )

    nc = _build_prog(Sn, KPp, live, BW)
    trace = bool(int(os.environ.get("KERNEL_TRACE", "0")))
    res = run_bass_kernel_spmd(nc, in_maps, list(range(NCORES)), trace=trace)
    LAST_EXEC_TIME_NS = res.exec_time_ns

    # ---- unshard ----
    out = np.zeros((Sn, E), np.float64)
    for c in range(NCORES):
        out += res.results[c]["outp"].T
    out = out.astype(np.float32).reshape(1, Sn, E)
    Kn = np.empty((1, KVH, Sn, D), np.float32)
    Vn = np.empty((1, KVH, Sn, D), np.float32)
    for g in range(KVH):
        r = res.results[2 * g]
        Kn[0, g][:, PERM] = r["knT"].T
        Vn[0, g] = r["vn"]
    return out, Kn, Vn


# revision 6
# speedup vs baseline: 1.1963x; 1.1963x over previous
"""Causal sliding-window GQA with 3D RoPE on 8 Trainium2 NeuronCores.

Sharding: tensor-parallel over the 16 query heads (2 per core); each core
also computes its group's kv head (duplicated across the 2 cores sharing
it). o_proj is row-parallel; the 8 partial outputs are summed on host.

Device kernel (per core, SPMD-uniform; all per-core differences are data):
  - Q/K/V projections as fp32r matmuls from host-pretransposed hidden^T.
  - 3D RoPE applied on-device with host-precomputed cos/sin tables; head
    dims are permuted [evens|odds] host-side (consistent for Q and K, so
    scores are unchanged) to make the rotation block-contiguous.
  - Scores are built transposed, S^T[k, q], so the exp output (ACT,
    PSUM->SBUF) feeds the P^T @ [V|1] matmul directly; the appended ones
    column yields the softmax denominators for free.
  - The sliding-window + frame-causal mask is precompiled host-side into a
    block-sparse live-tile list: fully masked key tiles are skipped, fully
    visible tiles run unmasked, and only partial tiles get an additive
    -1e30 bias region.
"""

import os
import sys

import ml_dtypes
import numpy as np

import concourse.bass as bass
import concourse.tile as tile
from concourse import mybir
from concourse.bass_utils import run_bass_kernel_spmd

sys.path.insert(0, os.path.dirname(os.path.abspath(__file__)))
from kernel_build import build as _build_prog  # noqa: E402

F32 = mybir.dt.float32
F32R = mybir.dt.float32r
AF = mybir.ActivationFunctionType
ALU = mybir.AluOpType

E = 1024
H = 16
KVH = 4
D = 64
HALF = 32
WINDOW = 32
SCALE = 0.125  # 1/sqrt(D)
NEG = np.float32(-1.0e30)
NCORES = 8

PERM = np.concatenate([np.arange(0, D, 2), np.arange(1, D, 2)])  # [evens|odds]

LAST_EXEC_TIME_NS = None


def kernel(hidden_states, past_k, past_v, wq, wk, wv, wo,
           new_t, new_d, new_b, past_t, past_d, past_b):
    global LAST_EXEC_TIME_NS
    hidden_states = np.asarray(hidden_states)
    B, Sn, _ = hidden_states.shape
    assert B == 1 and Sn % 128 == 0
    past_k = np.asarray(past_k, dtype=np.float32)
    past_v = np.asarray(past_v, dtype=np.float32)
    wq = np.asarray(wq, dtype=np.float32)
    wk = np.asarray(wk, dtype=np.float32)
    wv = np.asarray(wv, dtype=np.float32)
    wo = np.asarray(wo, dtype=np.float32)
    new_t = np.asarray(new_t)
    past_t = np.asarray(past_t)
    Lp = past_t.shape[0]

    # ---- mask structure (host) ----
    min_time = int(new_t.max()) - (WINDOW - 1)
    keep = int(np.searchsorted(past_t, min_time, side="left"))
    KP = Lp - keep
    KPp = ((KP + 127) // 128) * 128
    k0p = Lp - KPp
    pt_kept = past_t[k0p:Lp].astype(np.int64)
    sq_past = np.searchsorted(new_t, pt_kept, side="left").astype(np.int64)
    sq_past[pt_kept < min_time] = Sn  # pad keys: never visible
    sq_new = np.searchsorted(new_t, new_t, side="left").astype(np.int64)

    live = []  # (src, tile_idx, qlo, qpe, bias_off)
    mats = []
    boff = 0
    tiles = [("past", t, sq_past[t * 128 : (t + 1) * 128]) for t in range(KPp // 128)]
    tiles += [("new", t, sq_new[t * 128 : (t + 1) * 128]) for t in range(Sn // 128)]
    tiles_live = []
    for src, t, s in tiles:
        qlo = int(s.min())
        if qlo >= Sn:
            continue
        qpe = min(int(s.max()), Sn)
        tiles_live.append((src, t, qlo, qpe, s))
    tiles_live.sort(key=lambda x: x[2])
    assert tiles_live and tiles_live[0][2] == 0
    for src, t, qlo, qpe, s in tiles_live:
        w = qpe - qlo
        if w > 0:
            m = np.where(
                s[:, None] <= np.arange(qlo, qpe)[None, :], np.float32(0.0), NEG
            ).astype(np.float32)
            mats.append(m)
            live.append((src, t, qlo, qpe, boff))
            boff += w
        else:
            live.append((src, t, qlo, qpe, 0))
    BW = max(boff, 1)
    biases = np.zeros((128, BW), np.float32)
    if mats:
        biases[:, 0 : sum(m.shape[1] for m in mats)] = np.concatenate(mats, axis=1)

    # ---- RoPE trig tables (host, mirroring reference fp32 order) ----
    dim_idx = np.arange(HALF, dtype=np.float32)
    inv = (1.0 / (10000.0 ** (2.0 * dim_idx / D))).astype(np.float32)
    f_d = np.float32(10000.0 / 100.0)
    f_b = np.float32(10000.0 / 100.0)
    ang = (
        new_t.astype(np.float32)[:, None] * inv
        + new_d.astype(np.float32)[:, None] * inv * f_d
        + new_b.astype(np.float32)[:, None] * inv * f_b
    ).astype(np.float32)
    cos = np.cos(ang).astype(np.float32).T  # [32, Sn]
    sin = np.sin(ang).astype(np.float32).T
    t3 = np.concatenate([cos, sin, cos, sin], axis=0)  # [128, Sn]
    t4 = np.concatenate([sin, cos, sin, cos], axis=0)

    # ---- per-core inputs ----
    hiddenT = np.ascontiguousarray(hidden_states[0].T.astype(ml_dtypes.bfloat16))
    ident = np.eye(128, dtype=np.float32)
    KPa = max(KPp, 128)
    in_maps = []
    for c in range(NCORES):
        g = c // 2
        h0 = 4 * g + 2 * (c % 2)
        h1 = h0 + 1
        wq_cols = np.concatenate([h0 * D + PERM, h1 * D + PERM])
        wqp = np.ascontiguousarray(wq[:, wq_cols].astype(ml_dtypes.bfloat16))
        wkv = np.ascontiguousarray(
            np.concatenate(
                [wk[:, g * D + PERM], wv[:, g * D : (g + 1) * D]], axis=1
            ).astype(ml_dtypes.bfloat16)
        )
        wor = np.ascontiguousarray(
            np.concatenate(
                [wo[h0 * D : (h0 + 1) * D, :], wo[h1 * D : (h1 + 1) * D, :]]
            ).astype(ml_dtypes.bfloat16)
        )
        pkT = np.zeros((D, KPa), ml_dtypes.bfloat16)
        pvm = np.zeros((KPa, D), ml_dtypes.bfloat16)
        if KPp:
            pkT[:, 0:KPp] = past_k[0, g, k0p:Lp, :][:, PERM].T.astype(
                ml_dtypes.bfloat16
            )
            pvm[0:KPp] = past_v[0, g, k0p:Lp, :].astype(ml_dtypes.bfloat16)
        in_maps.append(
            dict(
                hT=hiddenT, wqp=wqp, wkv=wkv, wor=wor,
                pkT=np.ascontiguousarray(pkT), pv=np.ascontiguousarray(pvm),
                t3=t3, t4=t4, bias=biases, ident=ident,
            )
        )

    nc = _build_prog(Sn, KPp, live, BW)
    trace = bool(int(os.environ.get("KERNEL_TRACE", "0")))
    res = run_bass_kernel_spmd(nc, in_maps, list(range(NCORES)), trace=trace)
    LAST_EXEC_TIME_NS = res.exec_time_ns

    # ---- unshard ----
    out = np.zeros((Sn, E), np.float64)
    for c in range(NCORES):
        out += res.results[c]["outp"].T
    out = out.astype(np.float32).reshape(1, Sn, E)
    Kn = np.empty((1, KVH, Sn, D), np.float32)
    Vn = np.empty((1, KVH, Sn, D), np.float32)
    for g in range(KVH):
        r = res.results[2 * g]
        Kn[0, g][:, PERM] = r["knT"].T
        Vn[0, g] = r["vn"]
    return out, Kn, Vn
